# revision 8
# baseline (speedup 1.0000x reference)
"""PointConvolution (8-neighbor shifted diffs + 1x1 conv) as a single 3x3 conv,
run data-parallel across 8 TRN2 NeuronCores via Bass/Tile.

Math: out[o,h,w] = sum_k sum_c W[o,3k+c] * (xpad[c,h+ik,w+jk] - x[c,h,w]) + b[o]
    = sum_{c,i,j} K3[o,c,i,j] * xpad[c,h+i,w+j] + b[o]
  where K3 gets W at the 8 non-center taps and -sum(W over taps) at center.

Device scheme per core (2 images), v9 (fp16, interleaved 2x row tiling,
SBUF->SBUF j-replicas):
  - Output rows in chunks of 32 (TB=8 groups of G=4 rows). M=128 PSUM
    partitions = (g in 0..3, o in 0..31); contraction partitions
    18j + 6c + s for kernel column j, channel c, window row s in 0..5.
  - Host pre-gathers ONLY the j=0 im2row base [18, TB*Wp] fp16 per chunk
    (4.7MB/core HBM); the j=1,2 column-shift replicas are built on-device by
    two SBUF->SBUF DMA copies per chunk (no HBM traffic, and DMA descriptors
    don't care that the tile is only 18 partitions tall - engine copies
    would).
  - K=54 <= 64: PE runs in 64x128 row-tiled mode. EVEN chunks on SBUF
    partitions 0..53 / array tile (0,0); ODD chunks on 64..117 / tile
    (64,0). Matmul issue ALTERNATES parity per instruction so the two tiles
    stream concurrently (v8 grouped them per chunk and got no overlap from
    the in-order engine).
  - PSUM tiles [128, 2, 512] (2 banks); per q-step one per parity, bufs=4 =
    8 banks = double buffering. Drain (bias add + fp16 convert) splits
    engines: q=0,2 on DVE tensor_scalar_add, q=1,3 on ACT activation-add.
  - Queues: input (18-row base) on gpsimd/SWDGE; replica copies + outputs on
    the two HWDGE queues, cross-assigned (even copies + odd outputs on sync,
    odd copies + even outputs on scalar). One pair of software pipelining:
    inputs for pair p issue before compute of pair p-1, so copies never
    stall the queue behind an unmet input wait.
  - HBM traffic/core: 4.7MB in + 33.5MB out = 38.2MB -> ~107us roofline at
    358 GB/s. Host transposes + upcasts fp16->fp32 during unshard.
"""

import numpy as np

import concourse.bacc as bacc
import concourse.bass as bass
import concourse.tile as tile
from concourse import mybir
from concourse.bass_utils import run_bass_kernel_spmd

# Problem constants (hardcoded per harness contract)
B, C, H, W_DIM, OUT = 16, 3, 512, 512, 32
KS, P = 3, 1
NCORES = 8
NB = B // NCORES          # images per core = 2
Hp, Wp = H + 2 * P, W_DIM + 2 * P   # 514, 514

G = 4                     # output rows per matmul group
S = G + KS - 1            # input rows per group window = 6
TB = 8                    # groups per chunk (32 output rows)
CH = G * TB               # 32 output rows per chunk
NCHUNK = H // CH          # 16 chunks per image
NPAIR = NB * NCHUNK // 2  # chunk pairs per core = 16
K0 = C * S                # 18 base contraction partitions (j=0)
K = KS * K0               # 54 contraction partitions after replicas
M = G * OUT               # 128 output partitions
FW = TB * Wp              # 4112 free cols per contraction row
OBF = 2 * 4 * W_DIM       # 4096 free cols in the output tile
DR = 2                    # groups per PSUM tile / drain (2 banks)

F32 = mybir.dt.float32
F16 = mybir.dt.float16


def _coords():
    i, j = np.meshgrid(np.arange(KS), np.arange(KS))
    coords = np.dstack((i.reshape(-1), j.reshape(-1)))[0]
    return coords[np.any(coords != P, axis=1)]


def _build_weights(W, b):
    K3 = np.zeros((OUT, C, KS, KS), np.float32)
    Wr = W.reshape(OUT, 8, C)
    for k, (i, j) in enumerate(_coords()):
        K3[:, :, i, j] += Wr[:, k, :]
    K3[:, :, P, P] = -Wr.sum(axis=1)

    # wt[18j + 6c + s, 32g + o] = K3[o, c, s-g, j] when 0 <= s-g < KS
    wt = np.zeros((K, M), np.float32)
    for j in range(KS):
        for c in range(C):
            for s in range(S):
                for g in range(G):
                    i = s - g
                    if 0 <= i < KS:
                        wt[K0 * j + S * c + s, OUT * g: OUT * (g + 1)] = K3[:, c, i, j]
    bias = np.tile(b.astype(np.float32), G).reshape(M, 1)
    return wt.astype(np.float16), bias


def _build_xin(x):
    """[B,C,H,W] fp32 -> [B, NCHUNK, K0, TB*Wp] fp16 j=0 im2row over rows
    (padding embedded); j=1,2 replicas are built on-device."""
    x16 = np.ascontiguousarray(x, np.float32).astype(np.float16)
    xpad = np.pad(x16, ((0, 0), (0, 0), (P, P), (P, P)))  # [B,C,514,514]
    ch = np.arange(NCHUNK)[:, None, None]
    s = np.arange(S)[None, :, None]
    t = np.arange(TB)[None, None, :]
    rows = CH * ch + G * t + s                      # [NCHUNK, S, TB]
    big = xpad[:, :, rows, :]                       # [B, C, NCHUNK, S, TB, Wp]
    big = big.transpose(0, 2, 1, 3, 4, 5)           # [B, NCHUNK, C, S, TB, Wp]
    return np.ascontiguousarray(big).reshape(B, NCHUNK, K0, FW)


def _build_bass():
    # Bacc (not plain Bass): its compile() runs move_matmul_waits_to_ldweights
    # and generate_event_semaphores, required because TRN2 instructions take
    # at most one semaphore wait.
    nc = bacc.Bacc("TRN2")
    x_d = nc.declare_dram_parameter("xin", [NB, NCHUNK, K0, FW], F16, isOutput=False)
    wt_d = nc.declare_dram_parameter("wt", [K, M], F16, isOutput=False)
    b_d = nc.declare_dram_parameter("bias", [M, 1], F32, isOutput=False)
    out_d = nc.declare_dram_parameter("out", [NB, NCHUNK, M, OBF], F16, isOutput=True)

    with tile.TileContext(nc) as tc:
        with (
            tc.tile_pool(name="wpool", bufs=1) as wpool,
            tc.tile_pool(name="xpool", bufs=4) as xpool,
            tc.tile_pool(name="opool", bufs=3) as opool,
            tc.tile_pool(name="psum", bufs=2, space=bass.MemorySpace.PSUM) as ppool,
        ):
            # Weights at both row-tile partition bases (0 for tile (0,0),
            # 64 for tile (64,0)); bias indexes PSUM partitions (same both).
            wsb = wpool.tile([64 + K, M], F16)
            nc.scalar.dma_start(wsb[:K, :], wt_d[:])
            nc.scalar.dma_start(wsb[64:64 + K, :], wt_d[:])
            bsb = wpool.tile([M, 1], F32)
            nc.scalar.dma_start(bsb[:], b_d[:])

            def load_pair(pair):
                xin = xpool.tile([64 + K, FW], F16)
                for par in range(2):
                    ci = 2 * pair + par
                    base = 64 * par
                    src = bass.AP(x_d, ci * K0 * FW, [[FW, K0], [1, FW]])
                    nc.gpsimd.dma_start(xin[base: base + K0, :], src)
                return xin

            def process_pair(pair, xin):
                # j=1,2 replicas: SBUF->SBUF shifted copies (HWDGE queues,
                # split by parity). Matmul reads at most col FW-3, so the
                # 1-2 unwritten trailing replica cols never matter.
                for par in range(2):
                    base = 64 * par
                    qcp = nc.sync if par == 0 else nc.scalar
                    qcp.dma_start(xin[base + K0: base + 2 * K0, : FW - 1],
                                  xin[base: base + K0, 1:])
                    qcp.dma_start(xin[base + 2 * K0: base + 3 * K0, : FW - 2],
                                  xin[base: base + K0, 2:])

                obs = [opool.tile([M, OBF], F16, name=f"ob{par}") for par in range(2)]
                for q in range(TB // DR):          # 4 q-steps per chunk
                    pss = [ppool.tile([M, DR, W_DIM], F32, name=f"ps{par}") for par in range(2)]
                    for t2 in range(DR):
                        t = DR * q + t2
                        for par in range(2):       # alternate array tiles
                            base = 64 * par
                            nc.tensor.matmul(
                                pss[par][:, t2, :],
                                wsb[base: base + K, :],
                                xin[base: base + K, Wp * t: Wp * t + W_DIM],
                                start=True,
                                stop=True,
                                tile_position=(base, 0),
                            )
                    for par in range(2):
                        osl = obs[par][:, q * DR * W_DIM: (q + 1) * DR * W_DIM]
                        if q % 2 == 0:
                            nc.vector.tensor_scalar_add(osl, pss[par][:, :, :], bsb[:])
                        else:
                            nc.scalar.add(osl, pss[par][:, :, :], bsb[:])

                for par in range(2):
                    ci = 2 * pair + par
                    dst = bass.AP(out_d, ci * M * OBF, [[OBF, M], [1, OBF]])
                    qout = nc.scalar if par == 0 else nc.sync
                    qout.dma_start(dst, obs[par][:])

            # one-stage software pipeline: inputs run a pair ahead
            prev = load_pair(0)
            for pair in range(1, NPAIR):
                cur = load_pair(pair)
                process_pair(pair - 1, prev)
                prev = cur
            process_pair(NPAIR - 1, prev)
    nc.finalize()
    return nc


_NC_CACHE = None


def _get_nc():
    global _NC_CACHE
    if _NC_CACHE is None:
        _NC_CACHE = _build_bass()
    return _NC_CACHE


def kernel(x, W, b, trace=False, **trace_kw):
    xin = _build_xin(np.asarray(x, np.float32))
    wt, bias = _build_weights(np.asarray(W, np.float32), np.asarray(b, np.float32))
    in_maps = [
        {"xin": xin[NB * m: NB * (m + 1)], "wt": wt, "bias": bias}
        for m in range(NCORES)
    ]
    res = run_bass_kernel_spmd(
        _get_nc(), in_maps, list(range(NCORES)), trace=trace, **trace_kw
    )
    # Device layout [NB, NCHUNK, 32g+o, (half,t4,w)] -> [B, OUT, H, W]:
    # row = CH*chunk + 4*(4*half + t4) + g
    parts = []
    for m in range(NCORES):
        o = res.results[m]["out"].reshape(NB, NCHUNK, G, OUT, 2, 4, W_DIM)
        parts.append(o.transpose(0, 3, 1, 4, 5, 2, 6).reshape(NB, OUT, H, W_DIM))
    out = np.ascontiguousarray(np.concatenate(parts, axis=0)).astype(np.float32)
    if trace:
        kernel.last_results = res
    return out


# revision 9
# speedup vs baseline: 1.1438x; 1.1438x over previous
"""PointConvolution (8-neighbor shifted diffs + 1x1 conv) as a single 3x3 conv,
run data-parallel across 8 TRN2 NeuronCores via Bass/Tile.

Math: out[o,h,w] = sum_k sum_c W[o,3k+c] * (xpad[c,h+ik,w+jk] - x[c,h,w]) + b[o]
    = sum_{c,i,j} K3[o,c,i,j] * xpad[c,h+i,w+j] + b[o]
  where K3 gets W at the 8 non-center taps and -sum(W over taps) at center.

Device scheme per core (2 images), v10 (fp16, interleaved 2x row tiling,
pair-merged output DMAs):
  - Output rows in chunks of 32 (TB=8 groups of G=4 rows). M=128 PSUM
    partitions = (g in 0..3, o in 0..31); contraction partitions
    18j + 6c + s for kernel column j, channel c, window row s in 0..5.
  - Host pre-gathers each chunk's input window into [54, TB*Wp] fp16 im2row
    (rows AND the three j column-shifts materialized host-side; on-device
    replica building loses: ACT/DVE copies are lane-bound at 18/128
    partitions, and SBUF->SBUF DMA only engages ~4/16 SDMA engines for an
    18-partition tile - both measured slower than just loading 3x bytes).
  - K=54 <= 64: PE runs in 64x128 row-tiled mode. EVEN chunks on SBUF
    partitions 0..53 / array tile (0,0); ODD chunks on 64..117 / tile
    (64,0). Matmul issue ALTERNATES parity per instruction so the two array
    tiles stream concurrently (grouping by chunk gets no overlap from the
    in-order engine).
  - PSUM tiles [128, 2, 512] (2 banks), one per parity per q-step, bufs=2
    generations = all 8 banks. Drain (bias add + fp16 convert) splits
    engines: q=0,2 on DVE tensor_scalar_add, q=1,3 on ACT activation-add.
  - Output: ONE 2MB DMA per pair ([128, 2, 4096] fp16 ob tile = 16KB
    contiguous per partition -> line-rate descriptors), pairs alternate
    between the two HWDGE queues. Inputs: even chunks on gpsimd/SWDGE, odd
    chunks alternate sync/scalar to balance queue load.
  - HBM traffic/core: 14.2MB in + 33.5MB out = 47.7MB -> ~133us roofline at
    358 GB/s. Host transposes + upcasts fp16->fp32 during unshard.
"""

import numpy as np

import concourse.bacc as bacc
import concourse.bass as bass
import concourse.tile as tile
from concourse import mybir
from concourse.bass_utils import run_bass_kernel_spmd

# Problem constants (hardcoded per harness contract)
B, C, H, W_DIM, OUT = 16, 3, 512, 512, 32
KS, P = 3, 1
NCORES = 8
NB = B // NCORES          # images per core = 2
Hp, Wp = H + 2 * P, W_DIM + 2 * P   # 514, 514

G = 4                     # output rows per matmul group
S = G + KS - 1            # input rows per group window = 6
TB = 8                    # groups per chunk (32 output rows)
CH = G * TB               # 32 output rows per chunk
NCHUNK = H // CH          # 16 chunks per image
NPAIR = NB * NCHUNK // 2  # chunk pairs per core = 16
K0 = C * S                # 18 contraction partitions per kernel column j
K = KS * K0               # 54 contraction partitions total
M = G * OUT               # 128 output partitions
FW = TB * Wp              # 4112 free cols per contraction row
OBF = 2 * 4 * W_DIM       # 4096 free cols per chunk in the output tile
DR = 2                    # groups per PSUM tile / drain (2 banks)

F32 = mybir.dt.float32
F16 = mybir.dt.float16


def _coords():
    i, j = np.meshgrid(np.arange(KS), np.arange(KS))
    coords = np.dstack((i.reshape(-1), j.reshape(-1)))[0]
    return coords[np.any(coords != P, axis=1)]


def _build_weights(W, b):
    K3 = np.zeros((OUT, C, KS, KS), np.float32)
    Wr = W.reshape(OUT, 8, C)
    for k, (i, j) in enumerate(_coords()):
        K3[:, :, i, j] += Wr[:, k, :]
    K3[:, :, P, P] = -Wr.sum(axis=1)

    # wt[18j + 6c + s, 32g + o] = K3[o, c, s-g, j] when 0 <= s-g < KS
    wt = np.zeros((K, M), np.float32)
    for j in range(KS):
        for c in range(C):
            for s in range(S):
                for g in range(G):
                    i = s - g
                    if 0 <= i < KS:
                        wt[K0 * j + S * c + s, OUT * g: OUT * (g + 1)] = K3[:, c, i, j]
    bias = np.tile(b.astype(np.float32), G).reshape(M, 1)
    return wt.astype(np.float16), bias


def _build_xin(x):
    """[B,C,H,W] fp32 -> [B, NCHUNK, K, TB*Wp] fp16 im2row over rows, with the
    three j column-shift replicas stacked on the partition axis (padding
    embedded; 2 extra zero cols on the right so j-shifts never run off)."""
    x16 = np.ascontiguousarray(x, np.float32).astype(np.float16)
    xpad = np.pad(x16, ((0, 0), (0, 0), (P, P), (P, P + 2)))  # [B,C,514,516]
    ch = np.arange(NCHUNK)[:, None, None]
    s = np.arange(S)[None, :, None]
    t = np.arange(TB)[None, None, :]
    rows = CH * ch + G * t + s                      # [NCHUNK, S, TB]
    out = np.empty((B, NCHUNK, KS, C, S, TB, Wp), np.float16)
    for j in range(KS):
        rep = xpad[:, :, :, j:j + Wp]               # [B,C,514,514]
        big = rep[:, :, rows, :]                    # [B, C, NCHUNK, S, TB, Wp]
        out[:, :, j] = big.transpose(0, 2, 1, 3, 4, 5)
    return out.reshape(B, NCHUNK, K, FW)


def _build_bass():
    # Bacc (not plain Bass): its compile() runs move_matmul_waits_to_ldweights
    # and generate_event_semaphores, required because TRN2 instructions take
    # at most one semaphore wait.
    nc = bacc.Bacc("TRN2")
    x_d = nc.declare_dram_parameter("xin", [NB, NCHUNK, K, FW], F16, isOutput=False)
    wt_d = nc.declare_dram_parameter("wt", [K, M], F16, isOutput=False)
    b_d = nc.declare_dram_parameter("bias", [M, 1], F32, isOutput=False)
    out_d = nc.declare_dram_parameter(
        "out", [NB, NCHUNK // 2, M, 2, OBF], F16, isOutput=True
    )

    with tile.TileContext(nc) as tc:
        with (
            tc.tile_pool(name="wpool", bufs=1) as wpool,
            tc.tile_pool(name="xpool", bufs=4) as xpool,
            tc.tile_pool(name="opool", bufs=3) as opool,
            tc.tile_pool(name="psum", bufs=2, space=bass.MemorySpace.PSUM) as ppool,
        ):
            # Weights at both row-tile partition bases (0 for tile (0,0),
            # 64 for tile (64,0)); bias indexes PSUM partitions (same both).
            wsb = wpool.tile([64 + K, M], F16)
            nc.scalar.dma_start(wsb[:K, :], wt_d[:])
            nc.scalar.dma_start(wsb[64:64 + K, :], wt_d[:])
            bsb = wpool.tile([M, 1], F32)
            nc.scalar.dma_start(bsb[:], b_d[:])

            def load_pair(pair):
                xin = xpool.tile([64 + K, FW], F16)
                for par in range(2):
                    ci = 2 * pair + par
                    base = 64 * par
                    src = bass.AP(x_d, ci * K * FW, [[FW, K], [1, FW]])
                    if par == 0:
                        q = nc.gpsimd
                    else:
                        q = nc.sync if pair % 2 == 0 else nc.scalar
                    q.dma_start(xin[base: base + K, :], src)
                return xin

            def process_pair(pair, xin):
                ob = opool.tile([M, 2, OBF], F16)
                for q in range(TB // DR):          # 4 q-steps per chunk
                    pss = [ppool.tile([M, DR, W_DIM], F32, name=f"ps{par}")
                           for par in range(2)]
                    for t2 in range(DR):
                        t = DR * q + t2
                        for par in range(2):       # alternate array tiles
                            base = 64 * par
                            nc.tensor.matmul(
                                pss[par][:, t2, :],
                                wsb[base: base + K, :],
                                xin[base: base + K, Wp * t: Wp * t + W_DIM],
                                start=True,
                                stop=True,
                                tile_position=(base, 0),
                            )
                    for par in range(2):
                        osl = ob[:, par, q * DR * W_DIM: (q + 1) * DR * W_DIM]
                        if q % 2 == 0:
                            nc.vector.tensor_scalar_add(osl, pss[par][:, :, :], bsb[:])
                        else:
                            nc.scalar.add(osl, pss[par][:, :, :], bsb[:])

                dst = bass.AP(
                    out_d, pair * M * 2 * OBF, [[2 * OBF, M], [1, 2 * OBF]]
                )
                qout = nc.scalar if pair % 2 == 0 else nc.sync
                qout.dma_start(dst, ob[:])

            # one-stage software pipeline: inputs run a pair ahead
            prev = load_pair(0)
            for pair in range(1, NPAIR):
                cur = load_pair(pair)
                process_pair(pair - 1, prev)
                prev = cur
            process_pair(NPAIR - 1, prev)
    nc.finalize()
    return nc


_NC_CACHE = None


def _get_nc():
    global _NC_CACHE
    if _NC_CACHE is None:
        _NC_CACHE = _build_bass()
    return _NC_CACHE


def kernel(x, W, b, trace=False, **trace_kw):
    xin = _build_xin(np.asarray(x, np.float32))
    wt, bias = _build_weights(np.asarray(W, np.float32), np.asarray(b, np.float32))
    in_maps = [
        {"xin": xin[NB * m: NB * (m + 1)], "wt": wt, "bias": bias}
        for m in range(NCORES)
    ]
    res = run_bass_kernel_spmd(
        _get_nc(), in_maps, list(range(NCORES)), trace=trace, **trace_kw
    )
    # Device layout [NB, pairchunk, 32g+o, par, (half,t4,w)] -> [B, OUT, H, W]:
    # row = CH*(2*pc + par) + 4*(4*half + t4) + g
    parts = []
    for m in range(NCORES):
        o = res.results[m]["out"].reshape(NB, NCHUNK // 2, G, OUT, 2, 2, 4, W_DIM)
        parts.append(
            o.transpose(0, 3, 1, 4, 5, 6, 2, 7).reshape(NB, OUT, H, W_DIM)
        )
    out = np.ascontiguousarray(np.concatenate(parts, axis=0)).astype(np.float32)
    if trace:
        kernel.last_results = res
    return out


# revision 10
# speedup vs baseline: 1.1734x; 1.0258x over previous
"""PointConvolution (8-neighbor shifted diffs + 1x1 conv) as a single 3x3 conv,
run data-parallel across 8 TRN2 NeuronCores via Bass/Tile.

Math: out[o,h,w] = sum_k sum_c W[o,3k+c] * (xpad[c,h+ik,w+jk] - x[c,h,w]) + b[o]
    = sum_{c,i,j} K3[o,c,i,j] * xpad[c,h+i,w+j] + b[o]
  where K3 gets W at the 8 non-center taps and -sum(W over taps) at center.

Device scheme per core (2 images), v12 (bf16 matmuls, 4x32 PE row tiling,
minimal-HBM input):
  - Output rows in chunks of 32 = 8 groups of G=4 rows. M=128 PSUM
    partitions = (g, o). The j column shifts are NOT materialized: each
    group runs KS=3 bf16 matmuls that accumulate in PSUM, with the moving
    operand's column window shifted by j (the padded row is 514 wide, so
    [j : j+512] always fits). bf16 (not fp16!) because the PE streams bf16
    at 1 col/cycle; fp16 measured 743ns vs bf16's documented ~379ns per
    512-col matmul.
  - Row-window trick kills im2row row duplication: per chunk, quadrant q'
    (SBUF partitions 32q'..32q'+29) holds the 10 distinct input rows for
    groups t = 2q', 2q'+1 as partitions 3*rr + c. Group selection lives in
    the STATIONARY: w[tg][j] is [30, 128] with the 18 live rows placed at
    offset 12*tg, zeros elsewhere - so every matmul AP starts exactly at a
    32-aligned quadrant base, and the moving AP is the full quadrant.
  - PE runs 32x128 row-tiled: 4 tiles (0/32/64/96), one per quadrant,
    streaming concurrently. Matmul issue interleaves q' so all 4 tiles stay
    busy (also keeps the PE continuously fed - HAM throttles the array to
    half clock if it idles).
  - PSUM: ONE [128, 4, 512] tile (4 banks) per (par, tg); the 4 quadrant
    matmul groups write ps[:, q', :]; bufs=2 -> 8 banks ping-pong. ONE
    drain per (par, tg) (bias add + fp16 convert, 2048 cols - amortizes the
    per-instruction fixed cost), alternating DVE (tg=0) / ACT (tg=1), with
    a strided 3-dim dst AP since t = 2q' + tg interleaves in the row order.
  - Input: ONE gpsimd DMA per chunk-pair, [128, 2*514] bf16 (263KB, 2056B
    per-partition lines), prefetched 2 pairs deep -> 4.2MB/core.
    Output: ONE 2MB DMA per pair ([128, 2, 4096] fp16 = 16KB contiguous per
    partition), alternating between the two HWDGE queues.
  - HBM traffic/core: 4.2MB in + 33.5MB out = 37.7MB -> ~105us roofline at
    358 GB/s. Host transposes + upcasts fp16->fp32 during unshard.
"""

import ml_dtypes
import numpy as np

import concourse.bacc as bacc
import concourse.bass as bass
import concourse.tile as tile
from concourse import mybir
from concourse.bass_utils import run_bass_kernel_spmd

# Problem constants (hardcoded per harness contract)
B, C, H, W_DIM, OUT = 16, 3, 512, 512, 32
KS, P = 3, 1
NCORES = 8
NB = B // NCORES          # images per core = 2
Hp, Wp = H + 2 * P, W_DIM + 2 * P   # 514, 514

G = 4                     # output rows per matmul group
S = G + KS - 1            # input rows per group window = 6
TB = 8                    # groups per chunk (32 output rows)
CH = G * TB               # 32 output rows per chunk
NCHUNK = H // CH          # 16 chunks per image
NPC = NCHUNK // 2         # chunk pairs per image = 8
NPAIR = NB * NPC          # chunk pairs per core = 16
KQ = 30                   # contraction rows per quadrant (10 rows x 3 chan)
M = G * OUT               # 128 output partitions
OBF = TB * W_DIM          # 4096 free cols per chunk in the output tile

F32 = mybir.dt.float32
F16 = mybir.dt.float16
BF16 = mybir.dt.bfloat16
NP_BF16 = ml_dtypes.bfloat16


def _coords():
    i, j = np.meshgrid(np.arange(KS), np.arange(KS))
    coords = np.dstack((i.reshape(-1), j.reshape(-1)))[0]
    return coords[np.any(coords != P, axis=1)]


def _build_weights(W, b):
    K3 = np.zeros((OUT, C, KS, KS), np.float32)
    Wr = W.reshape(OUT, 8, C)
    for k, (i, j) in enumerate(_coords()):
        K3[:, :, i, j] += Wr[:, k, :]
    K3[:, :, P, P] = -Wr.sum(axis=1)

    # wts[tg, j, 12tg + 3s + c, 32g + o] = K3[o, c, s-g, j] when 0 <= s-g < KS
    wts = np.zeros((2, KS, KQ, M), np.float32)
    for tg in range(2):
        for j in range(KS):
            for s in range(S):
                for c in range(C):
                    for g in range(G):
                        i = s - g
                        if 0 <= i < KS:
                            wts[tg, j, 12 * tg + 3 * s + c,
                                OUT * g: OUT * (g + 1)] = K3[:, c, i, j]
    # -> [KQ, (tg,j) blocks of M cols] for a single DMA per quadrant base
    wt = wts.transpose(2, 0, 1, 3).reshape(KQ, 2 * KS * M)
    bias = np.tile(b.astype(np.float32), G).reshape(M, 1)
    return wt.astype(NP_BF16), bias


def _build_xin(x):
    """[B,C,H,W] fp32 -> [B, NPC, 128, 2*Wp] bf16: per chunk pair, partition
    32q' + 3rr + c holds padded row 32*chunk + 8q' + rr (rr in 0..9) of
    channel c, for both pair chunks side by side in the free dim."""
    x16 = np.ascontiguousarray(x, np.float32).astype(NP_BF16)
    xpad = np.pad(x16, ((0, 0), (0, 0), (P, P), (P, P)))  # [B,C,514,514]
    pc = np.arange(NPC)[:, None, None, None]
    qq = np.arange(4)[None, :, None, None]
    rr = np.arange(10)[None, None, :, None]
    par = np.arange(2)[None, None, None, :]
    rows = CH * (2 * pc + par) + 8 * qq + rr        # [NPC, 4, 10, 2]
    g = xpad[:, :, rows, :]                          # [B, C, NPC, 4, 10, 2, Wp]
    g = g.transpose(0, 2, 3, 4, 1, 5, 6)             # [B, NPC, 4, 10, C, 2, Wp]
    arr = np.zeros((B, NPC, 4, 32, 2, Wp), NP_BF16)
    arr[:, :, :, :KQ] = g.reshape(B, NPC, 4, KQ, 2, Wp)
    return arr.reshape(B, NPC, 128, 2 * Wp)


def _build_bass():
    # Bacc (not plain Bass): its compile() runs move_matmul_waits_to_ldweights
    # and generate_event_semaphores, required because TRN2 instructions take
    # at most one semaphore wait.
    nc = bacc.Bacc("TRN2")
    x_d = nc.declare_dram_parameter("xin", [NB, NPC, 128, 2 * Wp], BF16, isOutput=False)
    wt_d = nc.declare_dram_parameter("wt", [KQ, 2 * KS * M], BF16, isOutput=False)
    b_d = nc.declare_dram_parameter("bias", [M, 1], F32, isOutput=False)
    out_d = nc.declare_dram_parameter(
        "out", [NB, NPC, M, 2, OBF], F16, isOutput=True
    )

    with tile.TileContext(nc) as tc:
        with (
            tc.tile_pool(name="wpool", bufs=1) as wpool,
            tc.tile_pool(name="xpool", bufs=4) as xpool,
            tc.tile_pool(name="opool", bufs=3) as opool,
            tc.tile_pool(name="psum", bufs=2, space=bass.MemorySpace.PSUM) as ppool,
        ):
            # Stationaries replicated at all 4 quadrant bases; col block
            # (3*tg + j) * M selects the group-offset/shift variant.
            wsb = wpool.tile([96 + KQ, 2 * KS * M], BF16)
            for q in range(4):
                nc.scalar.dma_start(wsb[32 * q: 32 * q + KQ, :], wt_d[:])
            bsb = wpool.tile([M, 1], F32)
            nc.scalar.dma_start(bsb[:], b_d[:])

            def load_pair(pair):
                xin = xpool.tile([128, 2 * Wp], BF16)
                src = bass.AP(
                    x_d, pair * 128 * 2 * Wp, [[2 * Wp, 128], [1, 2 * Wp]]
                )
                nc.gpsimd.dma_start(xin[:], src)
                return xin

            def process_pair(pair, xin):
                ob = opool.tile([M, 2, OBF], F16)
                for par in range(2):
                    coff = par * Wp
                    for tg in range(2):
                        ps = ppool.tile([M, 4, W_DIM], F32)
                        for j in range(KS):
                            for q in range(4):     # round-robin the 4 tiles
                                nc.tensor.matmul(
                                    ps[:, q, :],
                                    wsb[32 * q: 32 * q + KQ,
                                        (KS * tg + j) * M: (KS * tg + j + 1) * M],
                                    xin[32 * q: 32 * q + KQ,
                                        coff + j: coff + j + W_DIM],
                                    start=(j == 0),
                                    stop=(j == KS - 1),
                                    tile_position=(32 * q, 0),
                                )
                        # one drain per (par, tg): dst rows t = 2q'+tg are
                        # 512-col blocks at stride 1024 in ob's chunk slab
                        dst = bass.AP(
                            ob.tensor,
                            ob.offset + par * OBF + tg * W_DIM,
                            [[2 * OBF, M], [2 * W_DIM, 4], [1, W_DIM]],
                        )
                        if tg == 0:
                            nc.vector.tensor_scalar_add(dst, ps[:, :, :], bsb[:])
                        else:
                            nc.scalar.add(dst, ps[:, :, :], bsb[:])

                odst = bass.AP(
                    out_d, pair * M * 2 * OBF, [[2 * OBF, M], [1, 2 * OBF]]
                )
                qout = nc.scalar if pair % 2 == 0 else nc.sync
                qout.dma_start(odst, ob[:])

            # two-stage input prefetch: loads run two pairs ahead
            tiles = [load_pair(0), load_pair(1)]
            for pair in range(2, NPAIR):
                tiles.append(load_pair(pair))
                process_pair(pair - 2, tiles.pop(0))
            process_pair(NPAIR - 2, tiles.pop(0))
            process_pair(NPAIR - 1, tiles.pop(0))
    nc.finalize()
    return nc


_NC_CACHE = None


def _get_nc():
    global _NC_CACHE
    if _NC_CACHE is None:
        _NC_CACHE = _build_bass()
    return _NC_CACHE


def kernel(x, W, b, trace=False, **trace_kw):
    xin = _build_xin(np.asarray(x, np.float32))
    wt, bias = _build_weights(np.asarray(W, np.float32), np.asarray(b, np.float32))
    in_maps = [
        {"xin": xin[NB * m: NB * (m + 1)], "wt": wt, "bias": bias}
        for m in range(NCORES)
    ]
    res = run_bass_kernel_spmd(
        _get_nc(), in_maps, list(range(NCORES)), trace=trace, **trace_kw
    )
    # Device layout [NB, pc, 32g+o, par, (t, w)] -> [B, OUT, H, W]:
    # row = 32*(2*pc + par) + 4*t + g
    parts = []
    for m in range(NCORES):
        o = res.results[m]["out"].reshape(NB, NPC, G, OUT, 2, TB, W_DIM)
        parts.append(
            o.transpose(0, 3, 1, 4, 5, 2, 6).reshape(NB, OUT, H, W_DIM)
        )
    out = np.ascontiguousarray(np.concatenate(parts, axis=0)).astype(np.float32)
    if trace:
        kernel.last_results = res
    return out


# revision 14
# speedup vs baseline: 1.2972x; 1.1056x over previous
"""PointConvolution (8-neighbor shifted diffs + 1x1 conv) as a single 3x3 conv,
run data-parallel across 8 TRN2 NeuronCores via Bass/Tile.

Math: out[o,h,w] = sum_k sum_c W[o,3k+c] * (xpad[c,h+ik,w+jk] - x[c,h,w]) + b[o]
    = sum_{c,i,j} K3[o,c,i,j] * xpad[c,h+i,w+j] + b[o]
  where K3 gets W at the 8 non-center taps and -sum(W over taps) at center.

Device scheme per core (2 images), v12 (bf16 matmuls, 4x32 PE row tiling,
minimal-HBM input):
  - Output rows in chunks of 32 = 8 groups of G=4 rows. M=128 PSUM
    partitions = (g, o). The j column shifts are NOT materialized: each
    group runs KS=3 bf16 matmuls that accumulate in PSUM, with the moving
    operand's column window shifted by j (the padded row is 514 wide, so
    [j : j+512] always fits). bf16 (not fp16!) because the PE streams bf16
    at 1 col/cycle; fp16 measured 743ns vs bf16's documented ~379ns per
    512-col matmul.
  - Row-window trick kills im2row row duplication: per chunk, quadrant q'
    (SBUF partitions 32q'..32q'+29) holds the 10 distinct input rows for
    groups t = 2q', 2q'+1 as partitions 3*rr + c. Group selection lives in
    the STATIONARY: w[tg][j] is [30, 128] with the 18 live rows placed at
    offset 12*tg, zeros elsewhere - so every matmul AP starts exactly at a
    32-aligned quadrant base, and the moving AP is the full quadrant.
  - PE runs 32x128 row-tiled: 4 tiles (0/32/64/96), one per quadrant,
    streaming concurrently. Matmul issue interleaves q' so all 4 tiles stay
    busy (also keeps the PE continuously fed - HAM throttles the array to
    half clock if it idles).
  - PSUM: one [128, 2, 512] tile (2 banks) per (par, tg, half); quadrants
    2h, 2h+1 write ps[:, q'%2, :]; bufs=4 -> 8 banks. The two halves of a
    step drain CONCURRENTLY on DVE (half 0) and ACT (half 1) - with a
    serial per-step drain the chain mm -> sem -> drain -> sem -> mm(k+2)
    was the pacing loop (v12 measured 2.7us/step vs 1.9us of stream). The
    drain dst is a strided 3-dim AP since t = 2q' + tg interleaves rows.
  - Input: ONE gpsimd DMA per chunk-pair, [128, 2*514] bf16 (263KB, 2056B
    per-partition lines), prefetched 2 pairs deep -> 4.2MB/core.
    Output: ONE 2MB DMA per pair ([128, 2, 4096] fp16 = 16KB contiguous per
    partition), alternating between the two HWDGE queues.
  - HBM traffic/core: 4.2MB in + 33.5MB out = 37.7MB -> ~105us roofline at
    358 GB/s. Host transposes + upcasts fp16->fp32 during unshard.
"""

import ml_dtypes
import numpy as np

import concourse.bacc as bacc
import concourse.bass as bass
import concourse.tile as tile
from concourse import mybir
from concourse.bass_utils import run_bass_kernel_spmd

# Problem constants (hardcoded per harness contract)
B, C, H, W_DIM, OUT = 16, 3, 512, 512, 32
KS, P = 3, 1
NCORES = 8
NB = B // NCORES          # images per core = 2
Hp, Wp = H + 2 * P, W_DIM + 2 * P   # 514, 514

G = 4                     # output rows per matmul group
S = G + KS - 1            # input rows per group window = 6
TB = 8                    # groups per chunk (32 output rows)
CH = G * TB               # 32 output rows per chunk
NCHUNK = H // CH          # 16 chunks per image
NPC = NCHUNK // 2         # chunk pairs per image = 8
NPAIR = NB * NPC          # chunk pairs per core = 16
KQ = 30                   # contraction rows per quadrant (10 rows x 3 chan)
M = G * OUT               # 128 output partitions
OBF = TB * W_DIM          # 4096 free cols per chunk in the output tile

F32 = mybir.dt.float32
F16 = mybir.dt.float16
BF16 = mybir.dt.bfloat16
NP_BF16 = ml_dtypes.bfloat16


def _coords():
    i, j = np.meshgrid(np.arange(KS), np.arange(KS))
    coords = np.dstack((i.reshape(-1), j.reshape(-1)))[0]
    return coords[np.any(coords != P, axis=1)]


def _build_weights(W, b):
    K3 = np.zeros((OUT, C, KS, KS), np.float32)
    Wr = W.reshape(OUT, 8, C)
    for k, (i, j) in enumerate(_coords()):
        K3[:, :, i, j] += Wr[:, k, :]
    K3[:, :, P, P] = -Wr.sum(axis=1)

    # wts[tg, j, 12tg + 3s + c, 32g + o] = K3[o, c, s-g, j] when 0 <= s-g < KS
    wts = np.zeros((2, KS, KQ, M), np.float32)
    for tg in range(2):
        for j in range(KS):
            for s in range(S):
                for c in range(C):
                    for g in range(G):
                        i = s - g
                        if 0 <= i < KS:
                            wts[tg, j, 12 * tg + 3 * s + c,
                                OUT * g: OUT * (g + 1)] = K3[:, c, i, j]
    # -> [KQ, (tg,j) blocks of M cols] for a single DMA per quadrant base
    wt = wts.transpose(2, 0, 1, 3).reshape(KQ, 2 * KS * M)
    bias = np.tile(b.astype(np.float32), G).reshape(M, 1)
    return wt.astype(NP_BF16), bias


def _build_xin(x):
    """[B,C,H,W] fp32 -> [B, NPC, 128, 2*Wp] bf16: per chunk pair, partition
    32q' + 3rr + c holds padded row 32*chunk + 8q' + rr (rr in 0..9) of
    channel c, for both pair chunks side by side in the free dim."""
    x16 = np.ascontiguousarray(x, np.float32).astype(NP_BF16)
    xpad = np.pad(x16, ((0, 0), (0, 0), (P, P), (P, P)))  # [B,C,514,514]
    pc = np.arange(NPC)[:, None, None, None]
    qq = np.arange(4)[None, :, None, None]
    rr = np.arange(10)[None, None, :, None]
    par = np.arange(2)[None, None, None, :]
    rows = CH * (2 * pc + par) + 8 * qq + rr        # [NPC, 4, 10, 2]
    g = xpad[:, :, rows, :]                          # [B, C, NPC, 4, 10, 2, Wp]
    g = g.transpose(0, 2, 3, 4, 1, 5, 6)             # [B, NPC, 4, 10, C, 2, Wp]
    arr = np.zeros((B, NPC, 4, 32, 2, Wp), NP_BF16)
    arr[:, :, :, :KQ] = g.reshape(B, NPC, 4, KQ, 2, Wp)
    return arr.reshape(B, NPC, 128, 2 * Wp)


def _build_bass():
    # Bacc (not plain Bass): its compile() runs move_matmul_waits_to_ldweights
    # and generate_event_semaphores, required because TRN2 instructions take
    # at most one semaphore wait.
    nc = bacc.Bacc("TRN2")
    x_d = nc.declare_dram_parameter("xin", [NB, NPC, 128, 2 * Wp], BF16, isOutput=False)
    wt_d = nc.declare_dram_parameter("wt", [KQ, 2 * KS * M], BF16, isOutput=False)
    b_d = nc.declare_dram_parameter("bias", [M, 1], F32, isOutput=False)
    out_d = nc.declare_dram_parameter(
        "out", [NB, NPC, M, 2, OBF], F16, isOutput=True
    )

    with tile.TileContext(nc) as tc:
        with (
            tc.tile_pool(name="wpool", bufs=1) as wpool,
            tc.tile_pool(name="xpool", bufs=4) as xpool,
            tc.tile_pool(name="opool", bufs=3) as opool,
            tc.tile_pool(name="psum", bufs=2, space=bass.MemorySpace.PSUM) as ppool,
        ):
            # Stationaries replicated at all 4 quadrant bases; col block
            # (3*tg + j) * M selects the group-offset/shift variant.
            wsb = wpool.tile([96 + KQ, 2 * KS * M], BF16)
            for q in range(4):
                nc.scalar.dma_start(wsb[32 * q: 32 * q + KQ, :], wt_d[:])
            bsb = wpool.tile([M, 1], F32)
            nc.scalar.dma_start(bsb[:], b_d[:])

            def load_pair(pair):
                xin = xpool.tile([128, 2 * Wp], BF16)
                src = bass.AP(
                    x_d, pair * 128 * 2 * Wp, [[2 * Wp, 128], [1, 2 * Wp]]
                )
                nc.gpsimd.dma_start(xin[:], src)
                return xin

            def process_pair(pair, xin):
                ob = opool.tile([M, 2, OBF], F16)
                for par in range(2):
                    coff = par * Wp
                    for tg in range(2):
                        pss = [ppool.tile([M, 2, W_DIM], F32, name=f"ps{h}")
                               for h in range(2)]
                        for j in range(KS):
                            for q in range(4):     # round-robin the 4 tiles
                                nc.tensor.matmul(
                                    pss[q // 2][:, q % 2, :],
                                    wsb[32 * q: 32 * q + KQ,
                                        (KS * tg + j) * M: (KS * tg + j + 1) * M],
                                    xin[32 * q: 32 * q + KQ,
                                        coff + j: coff + j + W_DIM],
                                    start=(j == 0),
                                    stop=(j == KS - 1),
                                    tile_position=(32 * q, 0),
                                )
                        # two concurrent drains per (par, tg): dst rows
                        # t = 2q'+tg are 512-col blocks at stride 1024
                        for h in range(2):
                            dst = bass.AP(
                                ob.tensor,
                                ob.offset + par * OBF + (4 * h + tg) * W_DIM,
                                [[2 * OBF, M], [2 * W_DIM, 2], [1, W_DIM]],
                            )
                            if h == 0:
                                nc.vector.tensor_scalar_add(
                                    dst, pss[h][:, :, :], bsb[:])
                            else:
                                nc.scalar.add(dst, pss[h][:, :, :], bsb[:])

                odst = bass.AP(
                    out_d, pair * M * 2 * OBF, [[2 * OBF, M], [1, 2 * OBF]]
                )
                qout = nc.scalar if pair % 2 == 0 else nc.sync
                qout.dma_start(odst, ob[:])

            # two-stage input prefetch: loads run two pairs ahead
            tiles = [load_pair(0), load_pair(1)]
            for pair in range(2, NPAIR):
                tiles.append(load_pair(pair))
                process_pair(pair - 2, tiles.pop(0))
            process_pair(NPAIR - 2, tiles.pop(0))
            process_pair(NPAIR - 1, tiles.pop(0))
    nc.finalize()
    return nc


_NC_CACHE = None


def _get_nc():
    global _NC_CACHE
    if _NC_CACHE is None:
        _NC_CACHE = _build_bass()
    return _NC_CACHE


def kernel(x, W, b, trace=False, **trace_kw):
    xin = _build_xin(np.asarray(x, np.float32))
    wt, bias = _build_weights(np.asarray(W, np.float32), np.asarray(b, np.float32))
    in_maps = [
        {"xin": xin[NB * m: NB * (m + 1)], "wt": wt, "bias": bias}
        for m in range(NCORES)
    ]
    res = run_bass_kernel_spmd(
        _get_nc(), in_maps, list(range(NCORES)), trace=trace, **trace_kw
    )
    # Device layout [NB, pc, 32g+o, par, (t, w)] -> [B, OUT, H, W]:
    # row = 32*(2*pc + par) + 4*t + g
    parts = []
    for m in range(NCORES):
        o = res.results[m]["out"].reshape(NB, NPC, G, OUT, 2, TB, W_DIM)
        parts.append(
            o.transpose(0, 3, 1, 4, 5, 2, 6).reshape(NB, OUT, H, W_DIM)
        )
    out = np.ascontiguousarray(np.concatenate(parts, axis=0)).astype(np.float32)
    if trace:
        kernel.last_results = res
    return out
